# revision 10
# baseline (speedup 1.0000x reference)
"""Trainium2 Bass kernel for a 2-layer GNN message-passing model (LightGCN-style).

Strategy (8 NeuronCores, SPMD):
  - Nodes (dest rows of the COO adjacency) are 1-D sharded: core c owns rows
    [c*12500, (c+1)*12500).  Edges are partitioned by destination row so the
    segment-sum is local to a core.
  - SpMM as staircase matmuls: for each 128-row destination block, accumulate
    over chunks of 128 edges   side_T[f, r] += feats[e, f] ^T* S[e, r]   where
    feats[e, :] = ego[cols[e], :] and S[e, r] = comb[e] * (rel_row[e] == r)
    built on VectorE with one fused tensor_scalar (is_equal, mult) against a
    constant iota tile.
  - Edge-feature rows are fetched with the Q7 dma_gather custom instruction
    (one instruction per strip-of-2-blocks x table-quarter, ~2K rows each;
    int16 indices address a 25000-row quarter of the fp16 table).
  - Dense per-layer work (W_gc / W_bi matmuls, bias, leaky-relu, row
    normalize) runs in fp32 on TensorE/VectorE/ScalarE.
  - Between layers the fp16 embedding table shard is AllGather'd (3.2->25.6MB).
  - Final users/pos/neg batch rows are routed to the owning core and gathered
    from a local fp32 [12500, 384] concat buffer via indirect DMA.
"""

import os
import sys

import numpy as np

sys.path.insert(0, "/opt/trn_rl_repo")

# ---------------------------------------------------------------- constants
N_USER = 50000
N_ITEM = 50000
N = N_USER + N_ITEM          # 100000 nodes
D = 128                      # embedding width (EMB + FEAT)
NLAYERS = 2
NCORES = 8
RPC = N // NCORES            # rows per core = 12500
BLK = 128                    # destination-row block (PSUM accumulation tile)
NQ = 4                       # table quarters (int16-addressable gather)
QROWS = N // NQ              # 25000
GSTRIP = 2                   # blocks per gather strip
NEG_SLOPE = 0.2
EPS = 1e-12


# ---------------------------------------------------------------- host prep
def _prep(inputs):
    users = np.asarray(inputs["users"]).astype(np.int64)
    pos_items = np.asarray(inputs["pos_items"]).astype(np.int64)
    neg_items = np.asarray(inputs["neg_items"]).astype(np.int64)
    user_emb = np.asarray(inputs["user_emb"], dtype=np.float32)
    item_emb1 = np.asarray(inputs["item_emb1"], dtype=np.float32)
    item_emb2 = np.asarray(inputs["item_emb2"], dtype=np.float32)
    feature = np.asarray(inputs["feature"], dtype=np.float32)
    w = np.asarray(inputs["w"], dtype=np.float32).reshape(1, 3)
    W_gc = np.ascontiguousarray(np.asarray(inputs["W_gc"], dtype=np.float32))
    b_gc = np.asarray(inputs["b_gc"], dtype=np.float32).reshape(NLAYERS, D)
    W_bi = np.ascontiguousarray(np.asarray(inputs["W_bi"], dtype=np.float32))
    b_bi = np.asarray(inputs["b_bi"], dtype=np.float32).reshape(NLAYERS, D)
    rows = np.asarray(inputs["adj_rows"]).astype(np.int64)
    cols = np.asarray(inputs["adj_cols"]).astype(np.int64)
    vals = np.asarray(inputs["adj_vals"], dtype=np.float32)

    nblk = (RPC + BLK - 1) // BLK                     # 98
    nstrip = (nblk + GSTRIP - 1) // GSTRIP            # 49

    ego0 = np.concatenate(
        [np.concatenate([user_emb, item_emb1], axis=0),
         np.concatenate([feature, item_emb2], axis=0)],
        axis=1,
    )  # [N, D] fp32
    ego0_16 = ego0.astype(np.float16)

    # ---- sort edges by (core, block, quarter)
    core_of = rows // RPC
    local = rows - core_of * RPC
    blk_of = local // BLK
    q_of = cols // QROWS
    key = (core_of * nblk + blk_of) * NQ + q_of
    order = np.argsort(key, kind="stable")
    key_s = key[order]
    rel_s = (local - blk_of * BLK)[order].astype(np.float32)
    qcol_s = (cols - q_of * QROWS)[order].astype(np.int16)
    vals_s = vals[order]

    # counts per (core, block, quarter) -> cross-core padded chunk counts
    cnt = np.bincount(key_s, minlength=NCORES * nblk * NQ).reshape(
        NCORES, nblk, NQ)
    nchq = np.ceil(cnt / 128).astype(np.int64).max(axis=0)  # [nblk, NQ]
    nch_blk = nchq.sum(axis=1)
    C = int(nch_blk.sum())

    # stream layout: strips of GSTRIP blocks; within a strip: quarters, and
    # within a quarter the member blocks' (padded) chunk runs back to back.
    choff = np.zeros((nblk, NQ), dtype=np.int64)
    strip_meta = []  # (chunk_off, nch_strip, [(q, qoff_in_strip, qchunks)], blocks)
    off = 0
    for s in range(nstrip):
        bs = list(range(s * GSTRIP, min((s + 1) * GSTRIP, nblk)))
        qruns = []
        s_off = off
        for q in range(NQ):
            qoff = off - s_off
            qc = 0
            for b in bs:
                choff[b, q] = off
                off += int(nchq[b, q])
                qc += int(nchq[b, q])
            qruns.append((q, qoff, qc))
        strip_meta.append((s_off, off - s_off, qruns, bs))
    assert off == C

    # per-core padded arrays
    E = C * 128
    relT = np.zeros((NCORES, 128, C), dtype=np.float32)
    valsT = np.zeros((NCORES, 128, C), dtype=np.float32)
    idx16 = np.zeros((NCORES, 16, 8 * C), dtype=np.int16)
    seg_start = np.searchsorted(key_s, np.arange(NCORES * nblk * NQ))
    seg_end = np.searchsorted(key_s, np.arange(NCORES * nblk * NQ) + 1)
    edge_off = choff * 128
    ar_e = np.arange(E)
    for c in range(NCORES):
        pr = np.zeros(E, dtype=np.float32)
        pv = np.zeros(E, dtype=np.float32)
        pq = np.zeros(E, dtype=np.int16)
        for b in range(nblk):
            for q in range(NQ):
                k = (c * nblk + b) * NQ + q
                s0, e0 = seg_start[k], seg_end[k]
                n = e0 - s0
                if n == 0:
                    continue
                d0 = edge_off[b, q]
                pr[d0:d0 + n] = rel_s[s0:e0]
                pv[d0:d0 + n] = vals_s[s0:e0]
                pq[d0:d0 + n] = qcol_s[s0:e0]
        relT[c] = pr.reshape(C, 128).T
        valsT[c] = pv.reshape(C, 128).T
        # 16-partition wrap for dma_gather: edge j -> [j%16, j//16]
        idx16[c][ar_e % 16, ar_e // 16] = pq
    idx16 = np.tile(idx16, (1, 8, 1))  # replicate to 128 partitions

    # ---- final batch gathers: route each batch element to its owning core
    def route(global_rows):
        owner = global_rows // RPC
        rel = (global_rows - owner * RPC).astype(np.int32)
        idx_lists, pos_lists = [], []
        for c in range(NCORES):
            m = np.where(owner == c)[0]
            idx_lists.append(rel[m])
            pos_lists.append(m)
        cap = max(max(len(x) for x in idx_lists), 1)
        cap = ((cap + 127) // 128) * 128
        idxT = np.zeros((NCORES, 128, cap // 128), dtype=np.int32)
        for c in range(NCORES):
            buf = np.zeros(cap, dtype=np.int32)
            buf[: len(idx_lists[c])] = idx_lists[c]
            idxT[c] = buf.reshape(cap // 128, 128).T
        return idxT, pos_lists, cap

    uT, upos, ucap = route(users)
    pT, ppos, pcap = route(N_USER + pos_items)
    nT, npos_, ncap = route(N_USER + neg_items)

    in_maps = []
    for c in range(NCORES):
        in_maps.append({
            "table0": ego0_16,
            "ego0_own": np.ascontiguousarray(ego0[c * RPC:(c + 1) * RPC]),
            "idx16": idx16[c],
            "relT": relT[c],
            "valsT": valsT[c],
            "w3": w,
            "Wg": W_gc,
            "Wb": W_bi,
            "bg": b_gc,
            "bb": b_bi,
            "uidx": uT[c],
            "pidx": pT[c],
            "nidx": nT[c],
        })
    meta = dict(C=C, nblk=nblk, strip_meta=strip_meta, nchq=nchq,
                ucap=ucap, pcap=pcap, ncap=ncap)
    routing = dict(upos=upos, ppos=ppos, npos=npos_)
    return in_maps, meta, routing


# ------------------------------------------------------------- device build
def _build(meta):
    import concourse.bass as bass
    import concourse.bacc as bacc
    import concourse.mybir as mybir
    import concourse.tile as tile
    from concourse import library_config
    from concourse.masks import make_identity

    f32 = mybir.dt.float32
    f16 = mybir.dt.float16
    i32 = mybir.dt.int32
    i16 = mybir.dt.int16
    Alu = mybir.AluOpType

    C = meta["C"]
    nblk = meta["nblk"]
    strip_meta = meta["strip_meta"]
    nchq = meta["nchq"]
    caps = [meta["ucap"], meta["pcap"], meta["ncap"]]
    max_strip_chunks = max(ns for (_, ns, _, _) in strip_meta)

    nc = bacc.Bacc("TRN2", num_devices=NCORES)

    # ---- kernel I/O
    table0 = nc.dram_tensor("table0", [N, D], f16, kind="ExternalInput")
    ego0_own = nc.dram_tensor("ego0_own", [RPC, D], f32, kind="ExternalInput")
    idx16_d = nc.dram_tensor("idx16", [128, 8 * C], i16, kind="ExternalInput")
    relT_d = nc.dram_tensor("relT", [128, C], f32, kind="ExternalInput")
    valsT_d = nc.dram_tensor("valsT", [128, C], f32, kind="ExternalInput")
    w3_d = nc.dram_tensor("w3", [1, 3], f32, kind="ExternalInput")
    Wg_d = nc.dram_tensor("Wg", [NLAYERS, D, D], f32, kind="ExternalInput")
    Wb_d = nc.dram_tensor("Wb", [NLAYERS, D, D], f32, kind="ExternalInput")
    bg_d = nc.dram_tensor("bg", [NLAYERS, D], f32, kind="ExternalInput")
    bb_d = nc.dram_tensor("bb", [NLAYERS, D], f32, kind="ExternalInput")
    idx_d = [
        nc.dram_tensor("uidx", [128, caps[0] // 128], i32, kind="ExternalInput"),
        nc.dram_tensor("pidx", [128, caps[1] // 128], i32, kind="ExternalInput"),
        nc.dram_tensor("nidx", [128, caps[2] // 128], i32, kind="ExternalInput"),
    ]
    out_d = [
        nc.dram_tensor("out_u", [caps[0], 3 * D], f32, kind="ExternalOutput"),
        nc.dram_tensor("out_p", [caps[1], 3 * D], f32, kind="ExternalOutput"),
        nc.dram_tensor("out_n", [caps[2], 3 * D], f32, kind="ExternalOutput"),
    ]

    with tile.TileContext(nc) as tc:
        with (
            tc.tile_pool(name="dram", bufs=1, space="DRAM") as dramp,
            tc.tile_pool(name="resident", bufs=1) as res,
            tc.tile_pool(name="vtmp", bufs=1) as vtmp,
            tc.tile_pool(name="feats", bufs=2) as featp,
            tc.tile_pool(name="idxp", bufs=2) as idxp,
            tc.tile_pool(name="sbuild", bufs=6) as sp,
            tc.tile_pool(name="dense", bufs=3) as dp,
            tc.tile_pool(name="psum_side", bufs=4, space="PSUM") as psp,
            tc.tile_pool(name="psum_misc", bufs=1, space="PSUM") as pmp,
        ):
            allcat = dramp.tile([RPC, 3 * D], f32, name="allcat")
            eg1_sh = dramp.tile([RPC, D], f16, name="eg1_sh")
            table1 = dramp.tile([N, D], f16, addr_space="Shared", name="table1")

            nc.gpsimd.load_library(library_config.mlp)

            # ---------------- constants / resident tiles
            iota16 = res.tile([128, 128], f16)
            iota_i = vtmp.tile([128, 128], i16)
            nc.gpsimd.iota(iota_i[:], pattern=[[1, 128]], base=0,
                           channel_multiplier=0)
            nc.vector.tensor_copy(iota16[:], iota_i[:])

            ident = res.tile([128, 128], f32)
            make_identity(nc, ident[:])

            ones1 = res.tile([1, 128], f32)
            nc.vector.memset(ones1[:], 1.0)

            rel_sb = res.tile([128, C], f32)
            nc.sync.dma_start(rel_sb[:], relT_d[:])

            w3_sb = res.tile([1, 3], f32)
            nc.sync.dma_start(w3_sb[:], w3_d[:])
            wb_ps = pmp.tile([128, 3], f32)
            nc.tensor.matmul(wb_ps[:], lhsT=ones1[:], rhs=w3_sb[:],
                             start=True, stop=True)
            wb = res.tile([128, 3], f32)
            nc.scalar.copy(wb[:], wb_ps[:])

            Wg_sb = [res.tile([D, D], f32, name=f"Wg{k}") for k in range(NLAYERS)]
            Wb_sb = [res.tile([D, D], f32, name=f"Wb{k}") for k in range(NLAYERS)]
            bc_sb = []
            for k in range(NLAYERS):
                nc.sync.dma_start(Wg_sb[k][:], Wg_d[k])
                nc.sync.dma_start(Wb_sb[k][:], Wb_d[k])
                bgk = vtmp.tile([1, D], f32, name=f"bgk{k}")
                bbk = vtmp.tile([1, D], f32, name=f"bbk{k}")
                nc.sync.dma_start(bgk[:], bg_d[k:k + 1, :])
                nc.sync.dma_start(bbk[:], bb_d[k:k + 1, :])
                bck = res.tile([1, D], f32, name=f"bc{k}")
                nc.vector.tensor_add(bck[:], bgk[:], bbk[:])
                bc_sb.append(bck)

            # ---------------- comb = w0*v + w1*v^2 + w2*v^6  (fp32, resident)
            v = vtmp.tile([128, C], f32)
            nc.sync.dma_start(v[:], valsT_d[:])
            v2 = vtmp.tile([128, C], f32)
            nc.vector.tensor_mul(v2[:], v[:], v[:])
            v6 = vtmp.tile([128, C], f32)
            nc.vector.tensor_mul(v6[:], v2[:], v2[:])        # v^4
            nc.vector.tensor_mul(v6[:], v6[:], v2[:])        # v^6
            comb = res.tile([128, C], f32)
            nc.vector.tensor_scalar(comb[:], v[:], wb[:, 0:1], None, op0=Alu.mult)
            nc.vector.tensor_scalar(v2[:], v2[:], wb[:, 1:2], None, op0=Alu.mult)
            nc.vector.tensor_scalar(v6[:], v6[:], wb[:, 2:3], None, op0=Alu.mult)
            nc.vector.tensor_add(comb[:], comb[:], v2[:])
            nc.vector.tensor_add(comb[:], comb[:], v6[:])

            # ---------------- ego0 -> allcat[:, 0:128]
            nc.sync.dma_start(allcat[:, 0:D], ego0_own[:])

            # ---------------- the two GNN layers
            for layer in range(NLAYERS):
                table = table0 if layer == 0 else table1
                for (s_off, s_nch, qruns, bs) in strip_meta:
                    if s_nch == 0:
                        continue
                    idxt = idxp.tile([128, 8 * max_strip_chunks], i16,
                                     tag="idxt")
                    nc.sync.dma_start(
                        idxt[:, :8 * s_nch],
                        idx16_d[:, 8 * s_off:8 * (s_off + s_nch)])
                    feats = featp.tile([128, max_strip_chunks * 128], f16,
                                       tag="feats")
                    for (q, qoff, qchunks) in qruns:
                        if qchunks == 0:
                            continue
                        nl = qchunks * 128
                        nc.gpsimd.dma_gather(
                            out_ap=feats[:, qoff * 128:(qoff + qchunks) * 128]
                            .rearrange("p (c e) -> p c e", e=D),
                            in_ap=table[q * QROWS:(q + 1) * QROWS, :],
                            idxs_ap=idxt[:, 8 * qoff:8 * (qoff + qchunks)],
                            num_idxs=nl,
                            num_idxs_reg=nl,
                            elem_size=D,
                            single_packet=False,
                        )
                    # spmm chunks, accumulated per destination block
                    ps_tiles = {}
                    done = {}
                    tot = {}
                    for b in bs:
                        ps_tiles[b] = psp.tile([128, 128], f32, tag="ps_side",
                                               name=f"ps_side_{layer}_{b}")
                        done[b] = 0
                        tot[b] = int(nchq[b].sum())
                        if tot[b] == 0:
                            nc.vector.memset(ps_tiles[b][:], 0.0)
                    for (q, qoff, qchunks) in qruns:
                        pos = qoff
                        for b in bs:
                            for j in range(int(nchq[b, q])):
                                ch = s_off + pos
                                S = sp.tile([128, 128], f16, tag="S")
                                nc.vector.tensor_scalar(
                                    S[:], iota16[:], rel_sb[:, ch:ch + 1],
                                    comb[:, ch:ch + 1],
                                    op0=Alu.is_equal, op1=Alu.mult)
                                nc.tensor.matmul(
                                    ps_tiles[b][:],
                                    lhsT=feats[:, pos * 128:(pos + 1) * 128],
                                    rhs=S[:],
                                    start=(done[b] == 0),
                                    stop=(done[b] == tot[b] - 1))
                                done[b] += 1
                                pos += 1
                    # dense epilogue per block
                    for b in bs:
                        r0 = b * BLK
                        rcnt = min(BLK, RPC - r0)
                        ps_side = ps_tiles[b]
                        sideT = dp.tile([128, 128], f32, tag="sideT")
                        nc.scalar.copy(sideT[:], ps_side[:])

                        ego_rows = dp.tile([128, 128], f32, tag="ego_rows")
                        if rcnt < 128:
                            nc.vector.memset(ego_rows[:], 0.0)
                        nc.sync.dma_start(
                            ego_rows[:rcnt, :],
                            allcat[r0:r0 + rcnt, layer * D:(layer + 1) * D])
                        ps_t = pmp.tile([128, 128], f32, tag="ps_t")
                        nc.tensor.transpose(ps_t[:], ego_rows[:], ident[:])
                        egoT = dp.tile([128, 128], f32, tag="egoT")
                        nc.scalar.copy(egoT[:], ps_t[:])
                        lhs_bi = dp.tile([128, 128], f32, tag="lhs_bi")
                        nc.vector.tensor_mul(lhs_bi[:], egoT[:], sideT[:])

                        ps_out = pmp.tile([128, 128], f32, tag="ps_out")
                        nc.tensor.matmul(ps_out[:], lhsT=sideT[:],
                                         rhs=Wg_sb[layer][:],
                                         start=True, stop=False)
                        nc.tensor.matmul(ps_out[:], lhsT=lhs_bi[:],
                                         rhs=Wb_sb[layer][:],
                                         start=False, stop=False)
                        nc.tensor.matmul(ps_out[:], lhsT=ones1[:],
                                         rhs=bc_sb[layer][:],
                                         start=False, stop=True)

                        tmp = dp.tile([128, 128], f32, tag="tmp")
                        nc.scalar.mul(tmp[:], ps_out[:], NEG_SLOPE)
                        act = dp.tile([128, 128], f32, tag="act")
                        nc.vector.tensor_tensor(act[:], ps_out[:], tmp[:],
                                                op=Alu.max)

                        sq = dp.tile([128, 128], f32, tag="sq")
                        ssq = dp.tile([128, 1], f32, tag="ssq")
                        nc.scalar.activation(
                            sq[:], act[:],
                            mybir.ActivationFunctionType.Square,
                            accum_out=ssq[:])
                        nrm = dp.tile([128, 1], f32, tag="nrm")
                        nc.scalar.sqrt(nrm[:], ssq[:])
                        nmx = dp.tile([128, 1], f32, tag="nmx")
                        nc.vector.tensor_scalar_max(nmx[:], nrm[:], EPS)
                        rinv = dp.tile([128, 1], f32, tag="rinv")
                        nc.vector.reciprocal(rinv[:], nmx[:])
                        egon = dp.tile([128, 128], f32, tag="egon")
                        nc.vector.tensor_scalar(egon[:], act[:], rinv[:, 0:1],
                                                None, op0=Alu.mult)
                        nc.sync.dma_start(
                            allcat[r0:r0 + rcnt,
                                   (layer + 1) * D:(layer + 2) * D],
                            egon[:rcnt, :])
                        if layer == 0:
                            egon16 = dp.tile([128, 128], f16, tag="egon16")
                            nc.scalar.copy(egon16[:], egon[:])
                            nc.sync.dma_start(eg1_sh[r0:r0 + rcnt, :],
                                              egon16[:rcnt, :])

                if layer == 0:
                    nc.gpsimd.collective_compute(
                        "AllGather",
                        Alu.bypass,
                        replica_groups=[list(range(NCORES))],
                        ins=[eg1_sh[:]],
                        outs=[table1[:]],
                    )

            # ---------------- final batch gathers from allcat
            for t in range(3):
                cap = caps[t]
                idx_sb = res.tile([128, cap // 128], i32, name=f"idx{t}")
                nc.sync.dma_start(idx_sb[:], idx_d[t][:])
                for chk in range(cap // 128):
                    g = sp.tile([128, 3 * D], f32, tag="outg")
                    nc.gpsimd.indirect_dma_start(
                        out=g[:],
                        out_offset=None,
                        in_=allcat[:],
                        in_offset=bass.IndirectOffsetOnAxis(
                            ap=idx_sb[:, chk:chk + 1], axis=0),
                    )
                    nc.sync.dma_start(out_d[t][chk * 128:(chk + 1) * 128, :],
                                      g[:])

    nc.compile()
    return nc


# ------------------------------------------------------------------- driver
LAST_RESULTS = None
LAST_NC = None


def kernel(**inputs):
    from concourse.bass_utils import run_bass_kernel_spmd

    in_maps, meta, routing = _prep(inputs)
    nc = _build(meta)
    trace = bool(int(os.environ.get("BASSK_TRACE", "0")))
    res = run_bass_kernel_spmd(nc, in_maps, core_ids=list(range(NCORES)),
                               trace=trace)
    global LAST_RESULTS, LAST_NC
    LAST_RESULTS = res
    LAST_NC = nc
    outs = res.results

    BATCH = sum(len(p) for p in routing["upos"])
    u_g = np.zeros((BATCH, 3 * D), np.float32)
    pos_g = np.zeros((BATCH, 3 * D), np.float32)
    neg_g = np.zeros((BATCH, 3 * D), np.float32)
    for c in range(NCORES):
        up, pp, np_ = routing["upos"][c], routing["ppos"][c], routing["npos"][c]
        u_g[up] = outs[c]["out_u"][: len(up)]
        pos_g[pp] = outs[c]["out_p"][: len(pp)]
        neg_g[np_] = outs[c]["out_n"][: len(np_)]
    return u_g, pos_g, neg_g
